# revision 7
# baseline (speedup 1.0000x reference)
"""Distributed autoregressive LSTM decoder on 8 TRN2 NeuronCores (v1.5).

Design (per core):
- vocab-sharded fc (4000 rows/core, v-ordered) on PE as fp32r matvec into 8
  PSUM banks; gates (512 rows/core) as TRUE-f32 matvec (accuracy) into bank 1.
- sigmoid computed as 0.5+0.5*tanh(0.5x) on ACT (tanh table is ~10x more
  accurate than the sigmoid table) + DVE affine.
- argmax: vector.max/max_index + PE transpose + eq/select ladder; the local
  top-1 candidate is then REFINED: its f32 weight column is gathered from
  DRAM (dynamic strided DMA) and re-dotted on PE in true-f32; the refined
  value is what gets exchanged cross-core. This resolves near-ties (the
  reference has a 1.7e-7 logit gap at step 104) that fp22 noise would flip.
- cross-core: remote_dma_broadcast (XOR-relative dests) under Switch(partition_id).
- token-dependent emb row gather: value_load + dynamic ds() DMA.
- PE warm-keeper: tiny filler matmuls keyed on mid-step semaphores keep the
  HAM clock gate at 8/8 (2.4 GHz) through the serial feedback gaps.
"""

from contextlib import ExitStack

import numpy as np

import concourse.bass as bass
import concourse.bacc as bacc
import concourse.mybir as mybir
from concourse import library_config
from concourse.bass_types import AP

F32 = mybir.dt.float32
F32R = mybir.dt.float32r
U32 = mybir.dt.uint32

H = 1024
V = 32000
NC = 8
VSH = V // NC          # 4000 vocab rows per core
NB = 8                 # psum banks used by fc
BN = VSH // NB         # 500 columns per bank
GATES = 512            # gate rows per core
NCH = 8                # hidden chunks (1024/128)
BIG = 1.0e9


def build_nc(T: int, with_tok_out: bool = True, with_fc_bias: bool = False,
             with_gate_bias: bool = False, static_gather: bool = False, skip_cand: bool = False):
    nc = bacc.Bacc("TRN2", debug=False)

    # ---------------- DRAM tensors ----------------
    d_fcw = nc.dram_tensor("fcw", [128, NCH * VSH], F32, kind="ExternalInput")
    d_whh = nc.dram_tensor("whh", [128, NCH * GATES], F32, kind="ExternalInput")
    d_wih = nc.dram_tensor("wih", [128, NCH * GATES], F32, kind="ExternalInput")
    d_emb = nc.dram_tensor("emb", [V, 2 * H], mybir.dt.bfloat16, kind="ExternalInput")
    d_h0 = nc.dram_tensor("h0", [128, 8], F32R, kind="ExternalInput")
    d_ident = nc.dram_tensor("ident", [128, 128], F32, kind="ExternalInput")
    d_iota = nc.dram_tensor("iota", [1, 256], F32, kind="ExternalInput")
    d_gbias = nc.dram_tensor("gbias", [1, GATES], F32, kind="ExternalInput")
    d_fcb = nc.dram_tensor("fcb", [1, VSH], F32R, kind="ExternalInput")
    d_one = nc.dram_tensor("one", [1, 1], F32R, kind="ExternalInput")
    d_out = nc.dram_tensor("out_logits", [T, VSH], F32, kind="ExternalOutput")
    d_tok = (
        nc.dram_tensor("tok_out", [T, 1], U32, kind="ExternalOutput")
        if with_tok_out
        else None
    )

    ctx = ExitStack()
    sb = lambda name, shape, dt=F32: ctx.enter_context(nc.sbuf_tensor(name, shape, dt))
    sem = lambda name: ctx.enter_context(nc.semaphore(name))

    # ---------------- SBUF ----------------
    fcw = sb("fcw_sb", [128, NCH * VSH], F32R)
    whh = sb("whh_sb", [128, NCH * GATES], F32)
    wih = sb("wih_sb", [128, NCH * GATES], F32)
    h_all = [sb(f"h_all{p}", [128, 8], F32R) for p in range(2)]
    x_raw = [sb(f"x_raw{p}", [128, 16], mybir.dt.bfloat16) for p in range(2)]
    x_rel = [sb(f"x_rel{p}", [128, 8]) for p in range(2)]
    cands = [sb(f"cands{p}", [128, 16]) for p in range(2)]
    gates_row = sb("gates_row", [1, GATES])
    gates_rd = sb("gates_rd", [128, 4])
    logits_sb = sb("logits_sb", [1, VSH])
    tanh_ifo = sb("tanh_ifo", [128, 3])
    sig_ifo = sb("sig_ifo", [128, 3])
    tanh_g = sb("tanh_g", [128, 1])
    tanh_c = sb("tanh_c", [128, 1])
    tmp1 = sb("tmp1", [128, 1])
    tmp2 = sb("tmp2", [128, 1])
    c_state = sb("c_state", [128, 1])
    h_send = sb("h_send", [128, 1], F32R)
    cand_send = sb("cand_send", [128, 2])
    logits_rd = sb("logits_rd", [128, 32])
    vals8 = sb("vals8", [128, 8])
    idx8 = sb("idx8", [128, 8], U32)
    pack = sb("pack", [128, 2])
    tp_sb = sb("tp_sb", [1, 256])
    vmax = sb("vmax", [1, 1])
    glob = sb("glob", [1, 128])
    globl = sb("globl", [1, 128])
    eqm = sb("eqm", [1, 128], U32)
    maskd = sb("maskd", [1, 128])
    maskl = sb("maskl", [1, 128])
    gidx = sb("gidx", [1, 1])
    qloc = sb("qloc", [1, 1])
    qloc_u = sb("qloc_u", [1, 1], U32)
    wcol = sb("wcol", [128, 8])
    refined = sb("refined", [1, 1])
    wmax = sb("wmax", [1, 1])
    eqw = sb("eqw", [1, 8], U32)
    maskw = sb("maskw", [1, 8])
    tokf = sb("tokf", [1, 1])
    tok_u = sb("tok_u", [1, 1], U32)
    x_row = sb("x_row", [1, H])
    ident = sb("ident_sb", [128, 128])
    iota = sb("iota_sb", [1, 256])
    gbias = sb("gbias_sb", [1, GATES])
    fcb = sb("fcb_sb", [1, VSH], F32R)
    ones11 = sb("ones11", [1, 1], F32R)
    bigc = sb("bigc", [1, 128])

    ps = ctx.enter_context(nc.psum_tensor("ps", [128, 4096], F32))
    # fc bank b: ps[0:1, 512b : 512b+500]
    # gates: ps[0:1, 512:1024] (bank 1, time-shared)
    # transpose out: ps[0:2, 0:256] (bank 0, time-shared)
    # refine slot: ps[0:1, 1012:1013] (bank 1 tail, drained before next gates)
    # filler slot: ps[0:1, 4084:4085] (bank 7 tail)

    # ---------------- semaphores ----------------
    s_pre = sem("s_pre")
    s_h = [sem(f"s_h{i}") for i in range(NC)]
    s_cand = [sem(f"s_cand{i}") for i in range(NC)]
    s_lsendh = sem("s_lsendh")
    s_lsendc = sem("s_lsendc")
    s_hready = sem("s_hready")
    s_cready = sem("s_cready")
    s_tokw = sem("s_tokw")
    s_gath = sem("s_gath")
    s_gat1 = sem("s_gat1")
    s_relu = sem("s_relu")
    s_pe_g = sem("s_pe_g")
    s_pe_fc = sem("s_pe_fc")
    s_pe_t = sem("s_pe_t")
    s_pe_r = sem("s_pe_r")     # refine matmuls done (+1/step)
    s_actgd = sem("s_actgd")   # ACT gates drain done (+1/step)
    s_adrn = sem("s_adrn")     # ACT fc-bank drains (+4/step)
    s_vdrn = sem("s_vdrn")     # DVE fc-bank drains (+4/step)
    s_gred = sem("s_gred")     # gates redistribute DMA (+16/step)
    s_lred = sem("s_lred")     # logits redistribute DMA (+16/step)
    s_out = sem("s_out")       # output row DMA (+16/step)
    s_tokd = sem("s_tokd")
    s_actp = sem("s_actp")
    s_actc = sem("s_actc")
    s_dvec = sem("s_dvec")
    s_dvepack = sem("s_dvepack")
    s_dvelad = sem("s_dvelad")
    s_cidx = sem("s_cidx")     # qloc_u ready (+1/step)
    s_wcol = sem("s_wcol")     # refine column gather DMA (+16/step)
    s_refd = sem("s_refd")     # refined value drained (+1/step)
    s_prep = sem("s_prep")

    PRE_TOTAL = 48 + 64 + 4
    GPRE = 64  # gpsimd SWDGE preloads on s_gath

    with nc.Block() as block:

        # ================= SYNC (HWDGE DMA) =================
        @block.sync
        def _(g: bass.BassEngine):
            g.dma_start(fcw[:, 0 : NCH * VSH // 2], d_fcw[:, 0 : NCH * VSH // 2].bitcast(F32R)).then_inc(s_pre, 16)
            g.dma_start(h_all[0][:, :], d_h0[:, :]).then_inc(s_pre, 16)
            g.dma_start(ident[:, :], d_ident[:, :]).then_inc(s_pre, 16)
            g.wait_ge(s_pre, PRE_TOTAL)
            for t in range(T):
                # gates row -> [128,4]
                g.wait_ge(s_actgd, t + 1)
                src = gates_row[0:1, :].rearrange("a (p q) -> a p q", p=128)
                g.dma_start(gates_rd[:, 0:4], src).then_inc(s_gred, 16)
                # logits row -> [125,32] in one DMA (v-order: v = 32p + j)
                g.wait_ge(s_adrn, 4 * (t + 1))
                g.wait_ge(s_vdrn, 4 * (t + 1))
                srcl = logits_sb[0:1, 0:4000].rearrange("a (p q) -> a p q", p=125)
                g.dma_start(logits_rd[0:125, 0:32], srcl).then_inc(s_lred, 16)
                # full logits row -> DRAM
                g.dma_start(d_out[t : t + 1, :], logits_sb[0:1, :]).then_inc(s_out, 16)
                # refine column gather (local top-1 candidate)
                if not skip_cand:
                    g.wait_ge(s_cidx, t + 1)
                    qv = g.value_load(qloc_u[0:1, 0:1])
                    with nc.allow_non_contiguous_dma(reason="4KB refine column gather"):
                        g.dma_start(
                            wcol[:, 0:8],
                            d_fcw.rearrange("p (c q) -> p q c", q=VSH)[:, bass.ds(qv, 1), :],
                        ).then_inc(s_wcol, 16)
                # token gather + debug out
                if t < T - 1:
                    if not skip_cand:
                        g.wait_ge(s_tokw, t + 1)
                    if static_gather:
                        g.dma_start(
                            x_raw[(t + 1) % 2][:, 0:16],
                            d_emb[0:1, :].rearrange("a (p c) -> (a p) c", p=128),
                        ).then_inc(s_gath, 16)
                    else:
                        tok = g.value_load(tok_u[0:1, 0:1])
                        g.dma_start(
                            x_raw[(t + 1) % 2][:, 0:16],
                            d_emb[bass.ds(tok, 1), :].rearrange("a (p c) -> (a p) c", p=128),
                        ).then_inc(s_gath, 16)
                    if with_tok_out and not skip_cand:
                        g.dma_start(d_tok[t : t + 1, :], tok_u[0:1, 0:1]).then_inc(
                            s_tokd, 16
                        )
            g.wait_ge(s_out, 16 * T)
            g.wait_ge(s_lred, 16 * T)
            if with_tok_out and T > 1 and not skip_cand:
                g.wait_ge(s_tokd, 16 * (T - 1))

        # ================= VECTOR (DVE) =================
        @block.vector
        def _(g: bass.BassVectorEngine):
            g.memset(c_state[:, :], 0.0).then_inc(s_pre, 1)
            g.memset(bigc[:, :], BIG).then_inc(s_pre, 1)
            g.memset(pack[:, :], 0.0).then_inc(s_pre, 1)
            g.memset(cand_send[:, :], 0.0).then_inc(s_pre, 1)
            g.wait_ge(s_pre, PRE_TOTAL)
            g.wait_ge(s_gath, GPRE)
            g.tensor_tensor(
                x_rel[0][:, :], x_raw[0][:, 0:8], x_raw[0][:, 8:16], mybir.AluOpType.add
            ).then_inc(s_relu, 1)
            for t in range(T):
                # ---- pointwise: sig_ifo = 0.5 + 0.5*tanh_ifo
                g.wait_ge(s_actp, t + 1)
                g.tensor_scalar(
                    sig_ifo[:, :], tanh_ifo[:, :], 0.5, 0.5,
                    op0=mybir.AluOpType.mult, op1=mybir.AluOpType.add,
                )
                g.drain()
                g.tensor_tensor(
                    tmp1[:, :], sig_ifo[:, 1:2], c_state[:, :], mybir.AluOpType.mult
                )
                g.tensor_tensor(
                    tmp2[:, :], sig_ifo[:, 0:1], tanh_g[:, :], mybir.AluOpType.mult
                )
                g.drain()
                g.tensor_tensor(
                    c_state[:, :], tmp1[:, :], tmp2[:, :], mybir.AluOpType.add
                ).then_inc(s_dvec, 1)
                g.wait_ge(s_actc, t + 1)
                if t > 0:
                    g.wait_ge(s_lsendh, 16 * t)
                g.tensor_tensor(
                    h_send[:, :], sig_ifo[:, 2:3], tanh_c[:, :], mybir.AluOpType.mult
                ).then_inc(s_hready, 1)
                # ---- fc bank drains 4..7 (PSUM -> logits_sb row)
                for b in range(4, NB):
                    g.wait_ge(s_pe_fc, NB * t + b + 1)
                    g.tensor_copy(
                        logits_sb[0:1, BN * b : BN * (b + 1)],
                        ps[0:1, 512 * b : 512 * b + BN],
                    ).then_inc(s_vdrn, 1)
                # ---- argmax ladder
                g.wait_ge(s_lred, 16 * (t + 1))
                g.max(vals8[0:125, :], logits_rd[0:125, 0:32])
                g.drain()
                g.max_index(idx8[0:125, :], vals8[0:125, :], logits_rd[0:125, 0:32])
                g.tensor_copy(pack[0:125, 0:1], vals8[0:125, 0:1])
                g.drain()
                g.tensor_copy(pack[0:125, 1:2], idx8[0:125, 0:1]).then_inc(s_dvepack, 1)
                g.wait_ge(s_pe_t, t + 1)
                g.tensor_copy(tp_sb[0:1, :], ps[0:1, 0:256]).then_inc(s_dvelad, 1)
                g.drain()
                g.tensor_reduce(
                    vmax[0:1, :], tp_sb[0:1, 0:125], mybir.AxisListType.X,
                    mybir.AluOpType.max,
                )
                g.tensor_add(glob[0:1, 0:125], tp_sb[0:1, 128:253], iota[0:1, 0:125])
                g.tensor_add(globl[0:1, 0:125], tp_sb[0:1, 128:253], iota[0:1, 128:253])
                g.drain()
                g.tensor_scalar(
                    eqm[0:1, 0:125], tp_sb[0:1, 0:125], vmax[0:1, 0:1],
                    scalar2=None, op0=mybir.AluOpType.is_equal,
                )
                g.drain()
                g.select(maskd[0:1, 0:125], eqm[0:1, 0:125], glob[0:1, 0:125], bigc[0:1, 0:125], add_drain=True)
                g.select(maskl[0:1, 0:125], eqm[0:1, 0:125], globl[0:1, 0:125], bigc[0:1, 0:125], add_drain=True)
                g.drain()
                g.tensor_reduce(
                    gidx[0:1, :], maskd[0:1, 0:125], mybir.AxisListType.X,
                    mybir.AluOpType.min,
                )
                g.tensor_reduce(
                    qloc[0:1, :], maskl[0:1, 0:125], mybir.AxisListType.X,
                    mybir.AluOpType.min,
                )
                g.drain()
                g.tensor_copy(qloc_u[0:1, :], qloc[0:1, :]).then_inc(s_cidx, 1)
                # ---- exchange refined local top-1
                if t > 0:
                    g.wait_ge(s_lsendc, 16 * t)
                g.wait_ge(s_refd, t + 1)
                g.tensor_copy(cand_send[0:1, 0:1], refined[0:1, :])
                g.drain()
                g.tensor_copy(cand_send[0:1, 1:2], gidx[0:1, :]).then_inc(s_cready, 1)
                # ---- winner (skip last step)
                if t < T - 1 and not skip_cand:
                    for i in range(NC):
                        g.wait_ge(s_cand[i], 2 * (t + 1))
                    cp = cands[t % 2]
                    g.tensor_reduce(
                        wmax[0:1, :], cp[0:1, 0:16:2], mybir.AxisListType.X,
                        mybir.AluOpType.max,
                    )
                    g.drain()
                    g.tensor_scalar(
                        eqw[0:1, :], cp[0:1, 0:16:2], wmax[0:1, 0:1],
                        scalar2=None, op0=mybir.AluOpType.is_equal,
                    )
                    g.drain()
                    g.select(maskw[0:1, :], eqw[0:1, :], cp[0:1, 1:16:2], bigc[0:1, 0:8], add_drain=True)
                    g.drain()
                    g.tensor_reduce(
                        tokf[0:1, :], maskw[0:1, :], mybir.AxisListType.X,
                        mybir.AluOpType.min,
                    )
                    g.drain()
                    g.tensor_copy(tok_u[0:1, :], tokf[0:1, :]).then_inc(s_tokw, 1)
                    g.wait_ge(s_gath, GPRE + 16 * (t + 1))
                    g.tensor_tensor(
                        x_rel[(t + 1) % 2][:, :], x_raw[(t + 1) % 2][:, 0:8],
                        x_raw[(t + 1) % 2][:, 8:16], mybir.AluOpType.add,
                    ).then_inc(s_relu, 1)
            g.wait_ge(s_lsendh, 16 * T)
            if T > 1:
                g.wait_ge(s_lsendc, 16 * (T - 1))

        # ================= SCALAR (ACT) =================
        @block.scalar
        def _(g: bass.BassScalarEngine):
            g.dma_start(whh[:, :], d_whh[:, :]).then_inc(s_pre, 16)
            g.dma_start(wih[:, :], d_wih[:, :]).then_inc(s_pre, 16)
            g.dma_start(fcw[:, NCH * VSH // 2 :], d_fcw[:, NCH * VSH // 2 :].bitcast(F32R)).then_inc(s_pre, 16)
            g.dma_start(iota[:, :], d_iota[:, :]).then_inc(s_pre, 16)
            g.wait_ge(s_pre, PRE_TOTAL)
            for t in range(T):
                # gates drain PSUM bank1 -> SBUF row
                g.wait_ge(s_pe_g, t + 1)
                g.activation(
                    gates_row[0:1, :], ps[0:1, 512:1024],
                    mybir.ActivationFunctionType.Copy,
                ).then_inc(s_actgd, 1)
                # pointwise activations: tanh(0.5x) for i,f,o; tanh(x) for g
                g.wait_ge(s_gred, 16 * (t + 1))
                g.activation(
                    tanh_ifo[:, :], gates_rd[:, 0:3], mybir.ActivationFunctionType.Tanh,
                    scale=0.5,
                )
                g.activation(
                    tanh_g[:, :], gates_rd[:, 3:4], mybir.ActivationFunctionType.Tanh
                ).then_inc(s_actp, 1)
                g.wait_ge(s_dvec, t + 1)
                g.activation(
                    tanh_c[:, :], c_state[:, :], mybir.ActivationFunctionType.Tanh
                ).then_inc(s_actc, 1)
                # fc bank drains 0..3
                for b in range(4):
                    g.wait_ge(s_pe_fc, NB * t + b + 1)
                    g.activation(
                        logits_sb[0:1, BN * b : BN * (b + 1)],
                        ps[0:1, 512 * b : 512 * b + BN],
                        mybir.ActivationFunctionType.Copy,
                    ).then_inc(s_adrn, 1)
                # refine drain
                if not skip_cand:
                    g.wait_ge(s_pe_r, t + 1)
                    g.activation(
                        refined[0:1, :], ps[0:1, 1012:1013],
                        mybir.ActivationFunctionType.Copy,
                    ).then_inc(s_refd, 1)
                else:
                    g.activation(
                        refined[0:1, :], vmax[0:1, :],
                        mybir.ActivationFunctionType.Copy,
                    ).then_inc(s_refd, 1)


        # ================= TENSOR (PE) =================
        @block.tensor
        def _(g: bass.BassTensorEngine):
            def filler(n=2):
                for _ in range(n):
                    g.matmul(
                        ps[0:1, 4084:4085],
                        h_all[0][:, 0:1].bitcast(F32),
                        h_all[0][:, 0:1].bitcast(F32),
                        start=True, stop=True, skip_group_check=True,
                    )

            g.wait_ge(s_pre, PRE_TOTAL)
            g.wait_ge(s_gath, GPRE)
            for t in range(T):
                pi_t = t % 2
                pi_n = (t + 1) % 2
                # ---- gates into psum bank 1 (true f32)
                g.wait_ge(s_relu, t + 1)
                if t > 0:
                    g.wait_ge(s_adrn, 4 * t)
                    g.wait_ge(s_vdrn, 4 * t)
                for c in range(NCH):
                    g.matmul(
                        ps[0:1, 512:1024],
                        x_rel[pi_t][:, c : c + 1],
                        wih[:, GATES * c : GATES * (c + 1)],
                        start=(c == 0),
                        stop=False,
                        skip_group_check=True,
                    )
                for c in range(NCH):
                    mm = g.matmul(
                        ps[0:1, 512:1024],
                        h_all[pi_t][:, c : c + 1].bitcast(F32),
                        whh[:, GATES * c : GATES * (c + 1)],
                        start=False,
                        stop=(c == NCH - 1) and not with_gate_bias,
                        skip_group_check=True,
                    )
                if with_gate_bias:
                    mm = g.matmul(
                        ps[0:1, 512:1024],
                        ones11[0:1, 0:1].bitcast(F32),
                        gbias[0:1, :],
                        start=False,
                        stop=True,
                        skip_group_check=True,
                    )
                mm.then_inc(s_pe_g, 1)
                # keep HAM warm through the pointwise gap
                g.wait_ge(s_gred, 16 * (t + 1))
                filler()
                g.wait_ge(s_actp, t + 1)
                filler()
                # ---- fc on h_{t+1}
                g.wait_ge(s_actgd, t + 1)       # bank1 drained
                if t > 0:
                    g.wait_ge(s_dvelad, t)      # bank0 transpose data consumed
                for c in range(NCH):
                    g.wait_ge(s_h[c], 2 * (t + 1))
                    for b in range(NB):
                        mm = g.matmul(
                            ps[0:1, 512 * b : 512 * b + BN],
                            h_all[pi_n][:, c : c + 1],
                            fcw[:, VSH * c + BN * b : VSH * c + BN * (b + 1)],
                            start=(c == 0),
                            stop=(c == NCH - 1) and not with_fc_bias,
                            skip_group_check=True,
                        )
                        if c == NCH - 1:
                            if with_fc_bias:
                                mm = g.matmul(
                                    ps[0:1, 512 * b : 512 * b + BN],
                                    ones11[0:1, 0:1],
                                    fcb[0:1, BN * b : BN * (b + 1)],
                                    start=False,
                                    stop=True,
                                    skip_group_check=True,
                                )
                            mm.then_inc(s_pe_fc, 1)
                # ---- transpose pack -> ps[0:1, 0:256] (bank 0, drained by DVE)
                g.wait_ge(s_dvepack, t + 1)
                g.wait_ge(s_adrn, 4 * t + 1)
                g.transpose(ps[0:1, 0:128], pack[:, 0:1], ident[:, :])
                g.transpose(ps[0:1, 128:256], pack[:, 1:2], ident[:, :]).then_inc(
                    s_pe_t, 1
                )
                # ---- refine: true-f32 re-dot of the local top-1 column
                if not skip_cand:
                    g.wait_ge(s_wcol, 16 * (t + 1))
                    for c in range(NCH):
                        mm = g.matmul(
                            ps[0:1, 1012:1013],
                            h_all[pi_n][:, c : c + 1].bitcast(F32),
                            wcol[:, c : c + 1],
                            start=(c == 0),
                            stop=(c == NCH - 1),
                            skip_group_check=True,
                        )
                    mm.then_inc(s_pe_r, 1)
                # warm fillers through the exchange/gather gap
                g.wait_ge(s_cready, t + 1)
                for _ in range(24):
                    g.matmul(
                        ps[0:1, 3584:4084],
                        h_all[0][:, 0:1],
                        fcw[:, 0:500],
                        start=True, stop=True, skip_group_check=True,
                    )
                if t < T - 1:
                    g.wait_ge(s_tokw, t + 1)
                    filler()
                    g.wait_ge(s_gath, GPRE + 16 * (t + 1))
                    filler()

        # ================= GPSIMD (remote + gather) =================
        @block.gpsimd
        def _(g: bass.BassGpSimd):
            g.load_library(library_config.remote_dma)
            g.dma_start(
                x_raw[0][:, 0:16],
                d_emb[0:1, :].rearrange("a (p c) -> (a p) c", p=128),
            ).then_inc(s_gath, 16)
            g.dma_start(gbias[:, :], d_gbias[:, :]).then_inc(s_gath, 16)
            g.dma_start(fcb[:, :], d_fcb[:, :]).then_inc(s_gath, 16)
            g.dma_start(ones11[:, :], d_one[:, :]).then_inc(s_gath, 16)
            g.wait_ge(s_pre, PRE_TOTAL)
            pid = g.partition_id()
            for case in g.Switch(pid, NC):
                prep = 0
                for t in range(T):
                    pi_t = t % 2
                    pi_n = (t + 1) % 2
                    g.remote_dma_broadcast(
                        out_ap=h_all[pi_n][:, case : case + 1],
                        in_ap=h_send[:, 0:1],
                        remote_sem=s_h[case],
                        local_sem=s_lsendh,
                        rdests=[(0, k) for k in range(NC)],
                    ).then_inc(s_prep, 1)
                    prep += 1
                    g.wait_ge(s_prep, prep)
                    g.wait_ge(s_hready, t + 1)
                    g.trigger_dma(1)
                    if t < T - 1 and not skip_cand:
                        g.remote_dma_broadcast(
                            out_ap=cands[pi_t][:, 2 * case : 2 * case + 2],
                            in_ap=cand_send[:, 0:2],
                            remote_sem=s_cand[case],
                            local_sem=s_lsendc,
                            rdests=[(0, k) for k in range(NC)],
                        ).then_inc(s_prep, 1)
                        prep += 1
                        g.wait_ge(s_prep, prep)
                        g.wait_ge(s_cready, t + 1)
                        g.trigger_dma(1)
                g.wait_ge(s_lsendh, 16 * T)
                if T > 1:
                    g.wait_ge(s_lsendc, 16 * (T - 1))

    nc.has_collectives = True
    ctx.close()
    nc.compile()
    return nc


# ======================= host-side prep =======================

def prep_core_inputs(inp: dict, T: int):
    """Returns list of per-core in_maps (np arrays). fc layout is v-ordered."""
    import jax
    import jax.numpy as jnp

    fc_W = np.asarray(inp["fc_W"], np.float32)
    W_ih = np.asarray(inp["W_ih"], np.float32)
    W_hh = np.asarray(inp["W_hh"], np.float32)
    embr = np.maximum(np.asarray(inp["emb"], np.float32), 0.0)
    e_hi = jnp.asarray(embr, dtype=jnp.bfloat16)
    e_lo = jnp.asarray(embr - np.asarray(e_hi.astype(jnp.float32)), dtype=jnp.bfloat16)
    emb_pair = np.empty((V, 128, 16), dtype=np.asarray(e_hi).dtype)
    emb_pair[:, :, 0:8] = np.asarray(e_hi).reshape(V, 128, 8)
    emb_pair[:, :, 8:16] = np.asarray(e_lo).reshape(V, 128, 8)
    emb = np.ascontiguousarray(emb_pair.reshape(V, 2 * H))
    b_ih = np.asarray(inp["b_ih"], np.float32)
    b_hh = np.asarray(inp["b_hh"], np.float32)
    fc_b = np.asarray(inp["fc_b"], np.float32)

    # h0 exactly as the f32 jax reference computes it
    cpu = jax.devices("cpu")[0]
    with jax.default_device(cpu):
        zc = jnp.concatenate([
            jnp.asarray(np.asarray(inp["z"], np.float32)),
            jnp.asarray(np.asarray(inp["c"], np.float32)),
        ])
        h0 = np.asarray(
            jnp.asarray(np.asarray(inp["l1_W"], np.float32)) @ zc
            + jnp.asarray(np.asarray(inp["l1_b"], np.float32))
        )
    h0_tile = np.ascontiguousarray(h0.reshape(8, 128).T)  # [128, 8]

    ident = np.eye(128, dtype=np.float32)

    d = np.arange(128)
    go = np.array([0, 1, 3, 2])  # col order i, f, o, g
    n = np.arange(GATES)

    maps = []
    for me in range(NC):
        gvoc = me * VSH + np.arange(VSH)       # v-order
        fcw = np.empty((128, NCH * VSH), np.float32)
        fcsel = fc_W[gvoc]  # [4000, 1024]
        for c in range(NCH):
            fcw[:, c * VSH : (c + 1) * VSH] = fcsel[:, 128 * c : 128 * (c + 1)].T
        grow = go[n % 4] * H + 128 * me + n // 4
        whh = np.empty((128, NCH * GATES), np.float32)
        wih = np.empty((128, NCH * GATES), np.float32)
        whsel = W_hh[grow]  # [512, 1024]
        wisel = W_ih[grow]
        for c in range(NCH):
            whh[:, c * GATES : (c + 1) * GATES] = whsel[:, 128 * c + d].T
            wih[:, c * GATES : (c + 1) * GATES] = wisel[:, d * 8 + c].T
        gbias = (b_ih + b_hh)[grow].reshape(1, GATES)
        fcb = fc_b[gvoc].reshape(1, VSH)
        iota = np.zeros((1, 256), np.float32)
        iota[0, :125] = me * VSH + np.arange(125) * 32
        iota[0, 128:253] = np.arange(125) * 32
        maps.append(
            dict(
                fcw=np.ascontiguousarray(fcw), whh=np.ascontiguousarray(whh),
                wih=np.ascontiguousarray(wih), emb=emb, h0=h0_tile, ident=ident,
                iota=iota, gbias=np.ascontiguousarray(gbias),
                fcb=np.ascontiguousarray(fcb), one=np.ones((1, 1), np.float32),
            )
        )
    return maps, None


def assemble_output(results, v_of_q, T: int):
    out = np.empty((T, 1, V), np.float32)
    for me, r in enumerate(results):
        out[:, 0, me * VSH : (me + 1) * VSH] = r["out_logits"]
    return out


# ======================= public entry point =======================

def kernel(**inputs):
    """Full-input distributed decoder: returns logits [T, 1, 32000] float32."""
    import numpy as np
    from concourse.bass_utils import run_bass_kernel_spmd

    T = int(inputs.get("max_length", 128))
    assert T == 128, f"kernel compiled for max_length=128, got {T}"

    inp = {k: (np.asarray(v) if hasattr(v, "shape") or not np.isscalar(v) else v)
           for k, v in inputs.items()}
    maps, _ = prep_core_inputs(inp, T)

    with_fc_bias = bool(np.any(np.asarray(inp["fc_b"], np.float32)))
    with_gate_bias = bool(
        np.any(np.asarray(inp["b_ih"], np.float32))
        or np.any(np.asarray(inp["b_hh"], np.float32))
    )
    nc = build_nc(T, with_fc_bias=with_fc_bias, with_gate_bias=with_gate_bias)
    res = run_bass_kernel_spmd(nc, maps, core_ids=list(range(NC)))
    return assemble_output(res.results, None, T)
